# revision 1
# baseline (speedup 1.0000x reference)
"""Conv2D 3x3 (B=32, C=128, H=W=56 -> OC=256) as a Bass/Tile kernel on 8 NeuronCores.

Strategy: data-parallel over batch (4 images per core), W/b replicated.
The conv is computed as 9 shift-matmuls accumulated in PSUM:
  out[oc, h, w] = sum_{kh,kw} W[oc, :, kh, kw] @ x_pad[:, h+kh, w+kw]
with x zero-padded to 58x58 on the host so every shifted window is a clean
strided view of one SBUF tile. Contraction dim C=128 sits on partitions,
OC=256 is two 128-row output tiles, and the 56x56 output plane is processed
in 7 blocks of 8 rows (N = 8*56 = 448 <= 512, one PSUM bank).

matmul dtype: float16 by default (full PE rate with fast weight load via
FWL; ~3e-4 rel err vs the fp32 reference given this problem's small dynamic
range). Set CONV_MM_DTYPE=f32r (~1.5e-4 err, ~11% slower), bf16, or f32 to
switch.
"""

import os

import numpy as np

import concourse.bacc as bacc
import concourse.mybir as mybir
import concourse.tile as tile
from concourse import bass_utils

B, C, H, W_SP = 32, 128, 56, 56
OC, KH, KW = 256, 3, 3
N_CORES = 8
B_PER = B // N_CORES            # 4 images per core
HP, WP = H + 2, W_SP + 2        # zero-padded spatial dims (58x58)
HWP = HP * WP                   # 3364
HWO = H * W_SP                  # 3136
ROWS_PER_TILE = 8               # output rows per matmul tile
N_TILE = ROWS_PER_TILE * W_SP   # 448 (<=512: one PSUM bank)
N_NT = H // ROWS_PER_TILE       # 7
OC_TILES = OC // 128            # 2

_NC_CACHE: dict[str, object] = {}


def _mm_mode() -> str:
    return os.environ.get("CONV_MM_DTYPE", "f16")


def _build_nc(mode: str):
    in_dt = {
        "bf16": mybir.dt.bfloat16,
        "f16": mybir.dt.float16,
        "f32r": mybir.dt.float32r,
        "f32": mybir.dt.float32,
    }[mode]
    nc = bacc.Bacc(
        "TRN2",
        target_bir_lowering=False,
        debug=False,
        enable_asserts=False,
        num_devices=N_CORES,
    )
    xp = nc.dram_tensor("xp", [B_PER, C, HWP], in_dt, kind="ExternalInput").ap()
    wt = nc.dram_tensor("wt", [C, KH * KW * OC], in_dt, kind="ExternalInput").ap()
    bias = nc.dram_tensor(
        "bias", [128, OC_TILES], mybir.dt.float32, kind="ExternalInput"
    ).ap()
    out = nc.dram_tensor(
        "out", [B_PER, OC, HWO], mybir.dt.float32, kind="ExternalOutput"
    ).ap()

    CHUNK_ROWS = ROWS_PER_TILE + KH - 1  # 10 padded rows per chunk (2-row halo)

    with tile.TileContext(nc) as tc:
        with (
            tc.tile_pool(name="xin", bufs=16) as xpool,
            tc.tile_pool(name="wpool", bufs=1) as wpool,
            tc.tile_pool(name="bpool", bufs=1) as bpool,
            tc.tile_pool(name="opool", bufs=4) as opool,
            tc.tile_pool(name="psum", bufs=4, space="PSUM") as pspool,
        ):
            # HAM warm-up: the PE clock-gate needs ~3.4us of sustained matmul
            # activity to lift to 2.4 GHz. Burn dummy matmuls on a zeroed tile
            # while the first DMAs are still in flight so the real stream
            # starts warm.
            wu = wpool.tile([C, 512], in_dt, tag="wu")
            nc.gpsimd.memset(wu[:], 0.0)
            psw = pspool.tile([128, 512], mybir.dt.float32, tag="ps")
            for i in range(7):
                nc.tensor.matmul(
                    psw[:],
                    wu[:, :128],
                    wu[:],
                    start=(i == 0),
                    stop=(i == 6),
                )

            # lead-in DMAs, finest first: tap 0, chunk 0, taps 1-2, then the
            # remaining tap rows — the k-th matmul of the first PSUM group
            # needs only tap k and chunk 0. Bias rides GpSimd (off the
            # critical Sync issue queue).
            wsb = wpool.tile([C, KH * KW, OC], in_dt, tag="wsb")
            wtv = wt.rearrange("c (k m) -> c k m", m=OC)
            xv0 = xp[0].rearrange("c (h w) -> c h w", w=WP)
            nc.sync.dma_start(wsb[:, 0, :], wtv[:, 0, :])
            xc0 = xpool.tile([C, CHUNK_ROWS, WP], in_dt, tag="xc")
            nc.sync.dma_start(xc0[:], xv0[:, :CHUNK_ROWS, :])
            nc.sync.dma_start(wsb[:, 1, :], wtv[:, 1, :])
            nc.sync.dma_start(wsb[:, 2, :], wtv[:, 2, :])
            for kg in range(1, KH):
                nc.sync.dma_start(
                    wsb[:, kg * KW : (kg + 1) * KW, :],
                    wtv[:, kg * KW : (kg + 1) * KW, :],
                )
            bsb = bpool.tile([128, OC_TILES], mybir.dt.float32, tag="bsb")
            nc.gpsimd.dma_start(bsb[:], bias[:])

            for img in range(B_PER):
                xv = xp[img].rearrange("c (h w) -> c h w", w=WP)
                for nt in range(N_NT):
                    r0 = nt * ROWS_PER_TILE
                    if img == 0 and nt == 0:
                        xc = xc0
                    else:
                        xc = xpool.tile([C, CHUNK_ROWS, WP], in_dt, tag="xc")
                        nc.sync.dma_start(xc[:], xv[:, r0 : r0 + CHUNK_ROWS, :])
                    for oc_t in range(OC_TILES):
                        # the very last group is split in two so its first
                        # half's output DMA overlaps the second half's
                        # matmuls (shortens the end-of-kernel drain wait)
                        is_last = (
                            img == B_PER - 1
                            and nt == N_NT - 1
                            and oc_t == OC_TILES - 1
                        )
                        subs = [(0, 4), (4, 4)] if is_last else [(0, ROWS_PER_TILE)]
                        for sr, nr in subs:
                            n_free = nr * W_SP
                            ps = pspool.tile(
                                [128, N_TILE], mybir.dt.float32, tag="ps"
                            )
                            ki = 0
                            for kh in range(KH):
                                for kw in range(KW):
                                    rhs = xc[
                                        :, sr + kh : sr + kh + nr, kw : kw + W_SP
                                    ]
                                    lhsT = wsb[
                                        :,
                                        kh * KW + kw,
                                        oc_t * 128 : (oc_t + 1) * 128,
                                    ]
                                    nc.tensor.matmul(
                                        ps[:, :n_free],
                                        lhsT,
                                        rhs,
                                        start=(ki == 0),
                                        stop=(ki == KH * KW - 1),
                                    )
                                    ki += 1
                            ot = opool.tile(
                                [128, N_TILE], mybir.dt.float32, tag="ot"
                            )
                            nc.scalar.activation(
                                ot[:, :n_free],
                                ps[:, :n_free],
                                mybir.ActivationFunctionType.Identity,
                                bias=bsb[:, oc_t : oc_t + 1],
                            )
                            col0 = nt * N_TILE + sr * W_SP
                            nc.sync.dma_start(
                                out[
                                    img,
                                    oc_t * 128 : (oc_t + 1) * 128,
                                    col0 : col0 + n_free,
                                ],
                                ot[:, :n_free],
                            )
    nc.compile()
    return nc


def _get_nc(mode: str):
    nc = _NC_CACHE.get(mode)
    if nc is None:
        nc = _build_nc(mode)
        _NC_CACHE[mode] = nc
    return nc


def kernel(x: np.ndarray, W: np.ndarray, b: np.ndarray) -> np.ndarray:
    mode = _mm_mode()
    x = np.asarray(x, dtype=np.float32)
    W = np.asarray(W, dtype=np.float32)
    b = np.asarray(b, dtype=np.float32)

    if mode == "bf16":
        import ml_dtypes

        in_np_dt = ml_dtypes.bfloat16
    elif mode == "f16":
        in_np_dt = np.float16
    else:
        in_np_dt = np.float32

    # Host-side layout prep: zero-pad x spatially, put the conv taps of W
    # into [tap, C, OC] (lhsT layout), stripe bias to [128, OC_TILES].
    xp = np.zeros((B, C, HP, WP), dtype=in_np_dt)
    xp[:, :, 1:-1, 1:-1] = x
    xp = xp.reshape(N_CORES, B_PER, C, HWP)
    # wt[c, k*OC + oc] = W[oc, c*9 + k]  (lhsT tap blocks, contiguous per c)
    wt = np.ascontiguousarray(
        W.reshape(OC, C, KH * KW).transpose(1, 2, 0).reshape(C, KH * KW * OC)
    ).astype(in_np_dt)
    bias = np.ascontiguousarray(b.reshape(OC_TILES, 128).T).astype(np.float32)

    nc = _get_nc(mode)
    in_maps = [
        {"xp": np.ascontiguousarray(xp[i]), "wt": wt, "bias": bias}
        for i in range(N_CORES)
    ]
    trace = os.environ.get("CONV_TRACE", "") not in ("", "0")
    try:
        res = bass_utils.run_bass_kernel_spmd(
            nc,
            in_maps,
            core_ids=list(range(N_CORES)),
            trace=trace,
        )
    except Exception:
        # transient device wedges (NRT_EXEC_UNIT_UNRECOVERABLE) have been
        # observed once; a fresh dispatch usually recovers
        import time

        time.sleep(2.0)
        res = bass_utils.run_bass_kernel_spmd(
            nc,
            in_maps,
            core_ids=list(range(N_CORES)),
            trace=trace,
        )
    kernel._last_results = res  # for test harness introspection
    out = np.stack([res.results[i]["out"] for i in range(N_CORES)])
    return out.reshape(B, OC, H, W_SP)

